# revision 23
# baseline (speedup 1.0000x reference)
"""BoundaryLoss kernel for Trainium2 (8 NeuronCores, data-parallel over batch).

Per (batch, waypoint): nearest of N=4096 boundary points (argmin euclidean),
dot(waypoint - closest_pt, closest_normal), exp_relu, mean over everything.

Per core: 4 batches x 2 chunks of 128 waypoints = 8 tiles of [128 wp, 4096 bp].

Engine plan per tile:
  PE    score[w,n] = wp.bp - 0.5*||bp||^2 as ONE bf16 matmul pass, K=21:
        fp32 inputs split into bf16 triples (hi/mid/lo); 6 cross terms per
        coordinate + 3 rows for the norm term. 1 cyc/col vs fp32's 4.
        8 matmuls of 512 cols into 4 PSUM bank-pairs P0..P3 [128,1024].
  DVE   fold1_0: max(P0[:,0:512], P0[:,512:1024]) -> folded[0:512] (PSUM in)
  ACT   copies P1..P3 -> SBUF (GPSIMD has no PSUM port)
  Pool  fold1_1..3 from the SBUF copy -> folded[512:2048]
  DVE   fold2 = tensor_tensor_reduce(max, accum=max): f2[1024] + exact max m
  DVE   max_index(f2, m) -> j; alias group of 4 original columns
  Pool  index convert; batched indirect gather of the 4-candidate payload rows
  DVE   exact re-verify of the 4 candidates (is_le tie-break -> lowest index),
        dot(delta, normal); ACT exp_relu; row-sum -> host mean.
"""

import numpy as np
import ml_dtypes

import concourse.bass as bass
import concourse.bacc as bacc
import concourse.bass_utils as bass_utils
import concourse.mybir as mybir
from concourse.tile import TileContext

B, W, N, D = 32, 256, 4096, 3
N_CORES = 8
BPC = B // N_CORES          # batches per core = 4
WCHUNKS = W // 128          # waypoint chunks per batch = 2
TILES = BPC * WCHUNKS       # 8 tiles per core
QUART = N // 4              # 1024 folded positions per tile
K = 21                      # 18 product rows + 3 norm rows

F32 = mybir.dt.float32
BF16 = mybir.dt.bfloat16
I32 = mybir.dt.int32
U32 = mybir.dt.uint32
ALU = mybir.AluOpType
ACTF = mybir.ActivationFunctionType

# term list: (wp piece, bp piece) covering the product to ~2^-24
TERMS = [(0, 0), (0, 1), (1, 0), (0, 2), (2, 0), (1, 1)]


def build_bass():
    nc = bacc.Bacc()

    # ---- DRAM I/O ----
    lhsTd = nc.dram_tensor("lhsTd", [K, BPC * W], BF16, kind="ExternalInput")
    rhsd = nc.dram_tensor("rhsd", [K, BPC * N], BF16, kind="ExternalInput")
    wpb = nc.dram_tensor("wpb", [128, TILES * D], F32, kind="ExternalInput")
    # candidate table: row (b*1024+j) holds bp/nrm for the 4 columns that
    # fold into j, ascending order
    gsrc = nc.dram_tensor("gsrc", [BPC * QUART, 8 * D], F32,
                          kind="ExternalInput")
    res = nc.dram_tensor("res", [128, 2], F32, kind="ExternalOutput")

    with TileContext(nc) as tc:
        with (
            tc.tile_pool(name="const", bufs=1) as cpool,
            tc.tile_pool(name="work", bufs=6) as wpool,
            tc.tile_pool(name="small", bufs=8) as spool,
            tc.tile_pool(name="psum", bufs=1, space="PSUM") as pp,
        ):
            # ---- input DMAs (HWDGE serializes: order matters); warm-ups
            # emitted early so their waits don't collapse onto later DMAs ----
            wa = cpool.tile([K, BPC * W], BF16)
            nc.sync.dma_start(out=wa[:], in_=lhsTd[:])
            actwarm = spool.tile([1, 1], F32, tag="actwarm")
            nc.scalar.activation(out=actwarm[:], in_=wa[0:1, 0:1],
                                 func=ACTF.Exp, scale=0.5)
            rb = cpool.tile([K, BPC * N], BF16)
            nc.sync.dma_start(out=rb[:, 0:2048], in_=rhsd[:, 0:2048])
            warm = pp.tile([128, QUART], F32, tag="p0")
            nc.tensor.matmul(out=warm[0:1, 0:1], lhsT=wa[:, 0:1],
                             rhs=wa[:, 1:2], start=True, stop=True)
            nc.tensor.matmul(out=warm[0:1, 1:2], lhsT=wa[:, 0:1],
                             rhs=wa[:, 2:3], start=True, stop=True)
            nc.sync.dma_start(out=rb[:, 2048:N], in_=rhsd[:, 2048:N])
            for b in range(1, BPC):
                nc.sync.dma_start(out=rb[:, b * N:(b + 1) * N],
                                  in_=rhsd[:, b * N:(b + 1) * N])
            wp_all = cpool.tile([128, TILES, D], F32)
            nc.sync.dma_start(out=wp_all[:], in_=wpb[:].rearrange(
                "p (t d) -> p t d", d=D))

            gall = cpool.tile([128, TILES, 8 * D], F32)
            dots = cpool.tile([128, TILES], F32)

            def verify(t0, t1):
                """Exact 4-candidate resolve: recompute squared distances,
                prefer the lower index on ties, write dot(delta, normal)."""
                n = t1 - t0
                ds, dots_c = [], []
                for ci in range(4):
                    bpC = gall[:, t0:t1, 2 * D * ci:2 * D * ci + D]
                    nrC = gall[:, t0:t1, 2 * D * ci + D:2 * D * ci + 2 * D]
                    sub = cpool.tile([128, n, D], F32, tag=f"sub{ci}_{t0}",
                                     name=f"sub{ci}_{t0}")
                    nc.vector.tensor_tensor(out=sub[:],
                                            in0=wp_all[:, t0:t1, :],
                                            in1=bpC, op=ALU.subtract)
                    sq = cpool.tile([128, n, D], F32, tag=f"sq{ci}_{t0}",
                                    name=f"sq{ci}_{t0}")
                    nc.vector.tensor_tensor(out=sq[:], in0=sub[:], in1=sub[:],
                                            op=ALU.mult)
                    dc = cpool.tile([128, n], F32, tag=f"d{ci}_{t0}",
                                    name=f"d{ci}_{t0}")
                    nc.vector.reduce_sum(out=dc[:], in_=sq[:],
                                         axis=mybir.AxisListType.X)
                    p = cpool.tile([128, n, D], F32, tag=f"p{ci}_{t0}",
                                   name=f"p{ci}_{t0}")
                    nc.vector.tensor_tensor(out=p[:], in0=sub[:], in1=nrC,
                                            op=ALU.mult)
                    dt = cpool.tile([128, n], F32, tag=f"dt{ci}_{t0}",
                                    name=f"dt{ci}_{t0}")
                    nc.vector.reduce_sum(out=dt[:], in_=p[:],
                                         axis=mybir.AxisListType.X)
                    ds.append(dc)
                    dots_c.append(dt)
                m01 = cpool.tile([128, n], U32, tag=f"m01_{t0}",
                                 name=f"m01_{t0}")
                nc.vector.tensor_tensor(out=m01[:], in0=ds[0][:],
                                        in1=ds[1][:], op=ALU.is_le)
                m23 = cpool.tile([128, n], U32, tag=f"m23_{t0}",
                                 name=f"m23_{t0}")
                nc.vector.tensor_tensor(out=m23[:], in0=ds[2][:],
                                        in1=ds[3][:], op=ALU.is_le)
                d01 = cpool.tile([128, n], F32, tag=f"d01_{t0}",
                                 name=f"d01_{t0}")
                nc.vector.tensor_tensor(out=d01[:], in0=ds[0][:],
                                        in1=ds[1][:], op=ALU.min)
                d23 = cpool.tile([128, n], F32, tag=f"d23_{t0}",
                                 name=f"d23_{t0}")
                nc.vector.tensor_tensor(out=d23[:], in0=ds[2][:],
                                        in1=ds[3][:], op=ALU.min)
                mf = cpool.tile([128, n], U32, tag=f"mf_{t0}",
                                name=f"mf_{t0}")
                nc.vector.tensor_tensor(out=mf[:], in0=d01[:], in1=d23[:],
                                        op=ALU.is_le)
                dot01 = cpool.tile([128, n], F32, tag=f"dot01_{t0}",
                                   name=f"dot01_{t0}")
                nc.vector.tensor_copy(dot01[:], dots_c[1][:])
                nc.vector.copy_predicated(dot01[:], m01[:], dots_c[0][:])
                dot23 = cpool.tile([128, n], F32, tag=f"dot23_{t0}",
                                   name=f"dot23_{t0}")
                nc.vector.tensor_copy(dot23[:], dots_c[3][:])
                nc.vector.copy_predicated(dot23[:], m23[:], dots_c[2][:])
                nc.vector.tensor_copy(dots[:, t0:t1], dot23[:])
                nc.vector.copy_predicated(dots[:, t0:t1], mf[:], dot01[:])

            # ---- main loop ----
            for t in range(TILES):
                b, wc = divmod(t, WCHUNKS)
                lhsT = wa[:, t * 128:(t + 1) * 128]
                ps = [pp.tile([128, QUART], F32, tag=f"p{k}",
                              name=f"p{k}_{t}")
                      for k in range(4)]
                # emission order P0, P1, P2, P3: ACT copies P0/P1 as soon as
                # their matmuls land; DVE then folds (P2-PSUM, sb0) and
                # (P3-PSUM, sb1). TensorTensor may read at most one PSUM
                # operand; Pool has no PSUM port; DMA can't read PSUM.
                for k in range(4):
                    for h in range(2):
                        col0 = b * N + k * QUART + h * 512
                        nc.tensor.matmul(out=ps[k][:, h * 512:(h + 1) * 512],
                                         lhsT=lhsT,
                                         rhs=rb[:, col0:col0 + 512],
                                         start=True, stop=True)
                sb0 = wpool.tile([128, QUART], F32, tag="sb0")
                nc.scalar.copy(out=sb0[:], in_=ps[0][:])
                sb1 = wpool.tile([128, QUART], F32, tag="sb1")
                nc.scalar.copy(out=sb1[:], in_=ps[1][:])
                # fold1 pairs (n, n+2048): folded[c]=max(P0[c],P2[c]),
                # folded[1024+c]=max(P1[c],P3[c]), both on DVE
                folded = wpool.tile([128, 2048], F32, tag="folded")
                nc.vector.tensor_tensor(out=folded[:, 0:QUART],
                                        in0=ps[2][:],
                                        in1=sb0[:], op=ALU.max)
                nc.vector.tensor_tensor(out=folded[:, QUART:2048],
                                        in0=ps[3][:],
                                        in1=sb1[:], op=ALU.max)
                # fold2 (j, j+1024): DVE takes [0:FD2], Pool emulates max as
                # relu(a-b)+b on [FD2:1024] (its TT/TS ops are legal)
                FD2 = 512
                f2 = wpool.tile([128, QUART], F32, tag="f2")
                nc.vector.tensor_tensor(out=f2[:, 0:FD2],
                                        in0=folded[:, 0:FD2],
                                        in1=folded[:, QUART:QUART + FD2],
                                        op=ALU.max)
                pw = QUART - FD2
                pdiff = wpool.tile([128, pw], F32, tag="pdiff")
                nc.gpsimd.tensor_tensor(out=pdiff[:],
                                        in0=folded[:, FD2:QUART],
                                        in1=folded[:, QUART + FD2:2048],
                                        op=ALU.subtract)
                prelu = wpool.tile([128, pw], F32, tag="prelu")
                nc.gpsimd.tensor_scalar(out=prelu[:], in0=pdiff[:],
                                        scalar1=0.0, scalar2=None,
                                        op0=ALU.max)
                nc.gpsimd.tensor_tensor(out=f2[:, FD2:QUART],
                                        in0=prelu[:],
                                        in1=folded[:, QUART + FD2:2048],
                                        op=ALU.add)
                v8 = spool.tile([128, 8], F32, tag="v8")
                nc.vector.max(out=v8[:], in_=f2[:])
                i8 = spool.tile([128, 8], U32, tag="i8")
                nc.vector.max_index(out=i8[:], in_max=v8[:], in_values=f2[:])
                # gather index = b*1024 + j
                idxf = spool.tile([128, 1], F32, tag="idxf")
                nc.gpsimd.tensor_scalar(out=idxf[:], in0=i8[:, 0:1],
                                        scalar1=float(b * QUART),
                                        scalar2=None, op0=ALU.add)
                idxi = spool.tile([128, 1], I32, tag="idxi")
                nc.gpsimd.tensor_copy(idxi[:], idxf[:])
                nc.gpsimd.indirect_dma_start(
                    out=gall[:, t, :], out_offset=None, in_=gsrc[:],
                    in_offset=bass.IndirectOffsetOnAxis(
                        ap=idxi[:, 0:1], axis=0))
                if t == 3:
                    verify(0, 4)
                elif t == 6:
                    verify(4, 7)

            def exp_relu_out(t0, t1, col):
                """exp_relu + row-sum of dots[:, t0:t1] -> res[:, col]."""
                n = t1 - t0
                e = cpool.tile([128, n], F32, tag=f"e{col}", name=f"e{col}")
                nc.scalar.activation(out=e[:], in_=dots[:, t0:t1],
                                     func=ACTF.Exp, scale=0.5)
                em1 = cpool.tile([128, n], F32, tag=f"em1{col}",
                                 name=f"em1{col}")
                nc.vector.tensor_scalar(out=em1[:], in0=e[:], scalar1=-1.0,
                                        scalar2=None, op0=ALU.add)
                gmask = cpool.tile([128, n], U32, tag=f"gm{col}",
                                   name=f"gm{col}")
                nc.vector.tensor_scalar(out=gmask[:], in0=dots[:, t0:t1],
                                        scalar1=0.0, scalar2=None,
                                        op0=ALU.is_gt)
                nc.vector.copy_predicated(em1[:], gmask[:], dots[:, t0:t1])
                sums = cpool.tile([128, 1], F32, tag=f"s{col}",
                                  name=f"s{col}")
                nc.vector.reduce_sum(out=sums[:], in_=em1[:],
                                     axis=mybir.AxisListType.X)
                nc.sync.dma_start(out=res[:, col:col + 1], in_=sums[:])

            # tiles 0..6 finish early; only tile 7's part rides the tail
            exp_relu_out(0, 7, 0)
            verify(7, TILES)
            exp_relu_out(7, TILES, 1)

    nc.finalize()
    return nc


_NC_CACHE = None


def _get_nc():
    global _NC_CACHE
    if _NC_CACHE is None:
        _NC_CACHE = build_bass()
    return _NC_CACHE


def _split3(x):
    """fp32 -> three bf16 pieces summing to x within ~2^-24 relative."""
    x = np.asarray(x, dtype=np.float32)
    a0 = x.astype(ml_dtypes.bfloat16).astype(np.float32)
    r = x - a0
    a1 = r.astype(ml_dtypes.bfloat16).astype(np.float32)
    a2 = (r - a1).astype(ml_dtypes.bfloat16).astype(np.float32)
    return a0, a1, a2


def make_in_maps(waypoints, boundarypoints, boundarynormals):
    waypoints = np.ascontiguousarray(waypoints, dtype=np.float32)
    boundarypoints = np.ascontiguousarray(boundarypoints, dtype=np.float32)
    boundarynormals = np.ascontiguousarray(boundarynormals, dtype=np.float32)
    in_maps = []
    for c in range(N_CORES):
        sl = slice(c * BPC, (c + 1) * BPC)
        wp_c = waypoints[sl]                      # [4, 256, 3]
        bp_c = boundarypoints[sl]                 # [4, 4096, 3]
        nrm_c = boundarynormals[sl]               # [4, 4096, 3]

        wsp = _split3(wp_c)                       # pieces [4,256,3]
        bsp = _split3(bp_c)
        s = (-0.5 * np.sum(bp_c.astype(np.float64) ** 2, axis=2)
             ).astype(np.float32)                 # [4,4096]
        ssp = _split3(s)

        lhsT = np.empty((K, BPC * W), dtype=np.float32)
        rhs = np.empty((K, BPC * N), dtype=np.float32)
        for d in range(D):
            for ti, (lw, rb_) in enumerate(TERMS):
                r = d * 6 + ti
                lhsT[r] = wsp[lw][:, :, d].reshape(BPC * W)
                rhs[r] = bsp[rb_][:, :, d].reshape(BPC * N)
        for k in range(3):
            lhsT[18 + k] = 1.0
            rhs[18 + k] = ssp[k].reshape(BPC * N)

        wpb = np.empty((128, TILES, D), dtype=np.float32)
        for t in range(TILES):
            b, wc = divmod(t, WCHUNKS)
            wpb[:, t, :] = wp_c[b, 128 * wc:128 * (wc + 1), :]

        # alias groups from fold1 (n, n+2048) then fold2 (j, j+1024):
        # {j, j+1024, j+2048, j+3072}
        j = np.arange(QUART)
        gsrc = np.empty((BPC, QUART, 8 * D), dtype=np.float32)
        for ci in range(4):
            idx = j + ci * QUART
            gsrc[:, :, 6 * ci:6 * ci + 3] = bp_c[:, idx]
            gsrc[:, :, 6 * ci + 3:6 * ci + 6] = nrm_c[:, idx]

        in_maps.append({
            "lhsTd": lhsT.astype(ml_dtypes.bfloat16),
            "rhsd": rhs.astype(ml_dtypes.bfloat16),
            "wpb": np.ascontiguousarray(wpb.reshape(128, TILES * D)),
            "gsrc": np.ascontiguousarray(gsrc.reshape(BPC * QUART, 8 * D)),
        })
    return in_maps


def run_on_device(waypoints, boundarypoints, boundarynormals, trace=False):
    nc = _get_nc()
    in_maps = make_in_maps(waypoints, boundarypoints, boundarynormals)
    out = bass_utils.run_bass_kernel_spmd(
        nc, in_maps, core_ids=list(range(N_CORES)), trace=trace)
    total = np.float64(0.0)
    for r in out.results:
        total += np.sum(r["res"], dtype=np.float64)
    value = np.float32(total / (B * W))
    return value, out


def kernel(waypoints, boundarypoints, boundarynormals):
    value, _ = run_on_device(waypoints, boundarypoints, boundarynormals)
    return np.asarray(value, dtype=np.float32)
